# revision 2
# baseline (speedup 1.0000x reference)
"""CSPN accelerate (3x3 per-pixel dynamic filter) on 8 Trainium2 NeuronCores.

out[b,0,h,w] = sum_{di,dj in 0..2} K[b,3*di+dj,h,w] * Xpad[b, h+di-1, w+dj-1]
with the center tap (di=dj=1) taking input0 instead of input.

Sharding: pure data-parallel over batch (16 samples -> 2 per core).

This version computes in fp16 (the op is a 9-term multiply-accumulate of
randn-scale data; fp16 products with fp32 PSUM accumulation land at ~7e-4
relative absmax error, far under the 2e-2 gate) which halves HBM traffic --
the binding resource for this memory-regime kernel.

Per-core kernel layout (per sample, per 126-row output tile):
  - x tile [128, 642] fp16: input rows [r-1, r+127) in partitions, zero
    columns 0/641 -> the three dj (column) shifts are free-dim slices.
  - x0 tile [128, 640] fp16: same row alignment as x.
  - ONE kernel tile [128, 3, 3, 640] fp16, filled by 3 DMAs (one per di,
    3 taps each, via a transposed DRAM access pattern), with tap rows
    loaded shifted by -di so the per-tap product k*x is computed aligned
    to *input* rows on DVE. These DMAs issue from the Pool engine (SWDGE)
    to keep the shared HWDGE issue pipe free for the x/x0/out DMAs.
  - 4 fused DVE multiplies produce all 9 tap products ([P,3,W] per dj
    with the x slice broadcast across di; the center tap uses x0).
  - matmuls vs constant shifted-identity matrices [128, th] realign the
    row (di) shifts and accumulate all 9 taps into PSUM on the Tensor
    engine; ACT copies PSUM (fp32) to SBUF as fp16 for a half-width
    output DMA (the host upcasts to fp32).
This reads every DRAM byte once (plus a 2/126 halo), at half width.
"""

import numpy as np

import concourse.bacc as bacc
import concourse.bass as bass
import concourse.mybir as mybir
import concourse.tile as tile
from concourse.bass_utils import run_bass_kernel_spmd

F16 = mybir.dt.float16
F32 = mybir.dt.float32

BS, KK, H, W = 16, 9, 480, 640
N_CORES = 8
BPC = BS // N_CORES          # samples per core
P = 128                      # SBUF partitions
TH = P - 2                   # output rows per tile (input tile covers TH+2 rows)
W_CHUNKS = ((0, 512), (512, 128))  # matmul moving-dim <= 512, one PSUM bank each


def build_module() -> bass.Bass:
    nc = bacc.Bacc()
    k_ext = nc.declare_dram_parameter("kern", [BPC, KK, H, W], F16, isOutput=False)
    x_ext = nc.declare_dram_parameter("x", [BPC, 1, H, W], F16, isOutput=False)
    x0_ext = nc.declare_dram_parameter("x0", [BPC, 1, H, W], F16, isOutput=False)
    out_ext = nc.declare_dram_parameter("out", [BPC, 1, H, W], F16, isOutput=True)

    # Shift matrices: M[di][g, h] = 1 iff g == h + di, so that
    # out[h] = sum_g M[g,h] * prod[g] picks input-row-aligned products back
    # into output rows.
    m_np = np.zeros((P, 3, TH), np.float16)
    for di in range(3):
        for h in range(TH):
            m_np[h + di, di, h] = 1.0
    m_dram = nc.inline_tensor(m_np, name="shiftm")

    row_tiles = [(r, min(TH, H - r)) for r in range(0, H, TH)]

    with tile.TileContext(nc) as tc:
        with (
            tc.tile_pool(name="consts", bufs=1) as cpool,
            tc.tile_pool(name="kpool", bufs=4) as kpool,
            tc.tile_pool(name="xpool", bufs=4) as xpool,
            tc.tile_pool(name="prodpool", bufs=4) as ppool,
            tc.tile_pool(name="opool", bufs=4) as opool,
            tc.tile_pool(name="psum", bufs=4, space="PSUM") as psumpool,
        ):
            mtile = cpool.tile([P, 3, TH], F16)
            nc.sync.dma_start(out=mtile[:], in_=m_dram[:])

            for b in range(BPC):
                for r, th in row_tiles:
                    _emit_tile(
                        nc, kpool, xpool, ppool, opool, psumpool, mtile,
                        k_ext, x_ext, x0_ext, out_ext, b, r, th,
                    )
    nc.finalize()
    return nc


def _emit_tile(nc, kpool, xpool, ppool, opool, psumpool, mtile,
               k_ext, x_ext, x0_ext, out_ext, b, r, th):
    lo = r - 1
    clo, chi = max(lo, 0), min(lo + P, H)
    top_edge = clo > lo          # first tile: image row -1 in partition 0
    bot_edge = chi < lo + P      # last tile: rows beyond 479

    # --- input tile: rows [r-1, r-1+P) of x, zero-padded columns at 0, 641
    xt = xpool.tile([P, W + 2], F16)
    # Engine APs must start at partition 0/32/64/96: pad memsets cover an
    # aligned 32-row block, the DMA load below overwrites the valid rows.
    nc.gpsimd.memset(xt[:, 0:1], 0.0)
    nc.gpsimd.memset(xt[:, W + 1:W + 2], 0.0)
    if top_edge:
        nc.gpsimd.memset(xt[0:32, :], 0.0)
    if bot_edge:
        nc.gpsimd.memset(xt[96:P, :], 0.0)
    nc.sync.dma_start(out=xt[clo - lo:chi - lo, 1:W + 1], in_=x_ext[b, 0, clo:chi, :])

    # --- center-tap replacement input0: same row alignment as x
    x0t = xpool.tile([P, W], F16, tag="x0t")
    if top_edge:
        nc.gpsimd.memset(x0t[0:32, :], 0.0)
    if bot_edge:
        nc.gpsimd.memset(x0t[96:P, :], 0.0)
    nc.sync.dma_start(out=x0t[clo - lo:chi - lo, :], in_=x0_ext[b, 0, clo:chi, :])

    # --- kernel taps: one [P, 3, 3, W] tile (partition=row, di, dj, w),
    # filled by one DMA per di with rows shifted by -di. Issued via the Pool
    # engine's SWDGE path to keep HWDGE free for the x/x0/out DMAs.
    kt = kpool.tile([P, 3, 3, W], F16)
    for di in range(3):
        klo = r - di
        kclo, kchi = max(klo, 0), min(klo + P, H)
        if kclo > klo:
            nc.gpsimd.memset(kt[0:32, di], 0.0)
        if kchi < klo + P:
            nc.gpsimd.memset(kt[96:P, di], 0.0)
        nc.gpsimd.dma_start(
            out=kt[kclo - klo:kchi - klo, di],
            in_=k_ext[b, 3 * di:3 * di + 3, kclo:kchi, :].transpose([1, 0, 2]),
        )

    # --- fused per-dj product batches on DVE: prod[p, dj, di, w] =
    # kt[p, di, dj, w] * x[p, w+dj] (x slice broadcast across di). The
    # center tap (di=1, dj=1) instead multiplies x0.
    prod = ppool.tile([P, 3, 3, W], F16)
    for dj in (0, 2):
        nc.vector.tensor_tensor(
            out=prod[:, dj],
            in0=kt[:, :, dj],
            in1=xt[:, dj:W + dj].unsqueeze(1).broadcast_to([P, 3, W]),
            op=mybir.AluOpType.mult,
        )
    nc.vector.tensor_tensor(
        out=prod[:, 1, 0::2],
        in0=kt[:, 0::2, 1],
        in1=xt[:, 1:W + 1].unsqueeze(1).broadcast_to([P, 2, W]),
        op=mybir.AluOpType.mult,
    )
    nc.vector.tensor_tensor(
        out=prod[:, 1, 1],
        in0=kt[:, 1, 1],
        in1=x0t[:],
        op=mybir.AluOpType.mult,
    )

    # --- row-shift realignment + 9-tap accumulation in PSUM on PE
    psum_t = psumpool.tile([P, W], F32)
    for di in range(3):
        for dj in range(3):
            tap = 3 * di + dj
            for c0, cn in W_CHUNKS:
                nc.tensor.matmul(
                    out=psum_t[:th, c0:c0 + cn],
                    lhsT=mtile[:, di, 0:th],
                    rhs=prod[:, dj, di, c0:c0 + cn],
                    start=(tap == 0),
                    stop=(tap == 8),
                )

    # --- PSUM (fp32) -> SBUF fp16 (on ACT) -> DRAM
    osb = opool.tile([P, W], F16, tag="osb")
    nc.scalar.copy(out=osb[:th, :], in_=psum_t[:th, :])
    nc.scalar.dma_start(out=out_ext[b, 0, r:r + th, :], in_=osb[:th, :])


_NC_CACHE = None


def _get_module():
    global _NC_CACHE
    if _NC_CACHE is None:
        _NC_CACHE = build_module()
    return _NC_CACHE


def kernel(**inputs: np.ndarray) -> np.ndarray:
    kern = np.asarray(inputs["kernel"], dtype=np.float32).astype(np.float16)
    x = np.asarray(inputs["input"], dtype=np.float32).astype(np.float16)
    x0 = np.asarray(inputs["input0"], dtype=np.float32).astype(np.float16)
    assert kern.shape == (BS, KK, H, W), kern.shape

    nc = _get_module()
    in_maps = [
        {
            "kern": np.ascontiguousarray(kern[c * BPC:(c + 1) * BPC]),
            "x": np.ascontiguousarray(x[c * BPC:(c + 1) * BPC]),
            "x0": np.ascontiguousarray(x0[c * BPC:(c + 1) * BPC]),
        }
        for c in range(N_CORES)
    ]
    res = run_bass_kernel_spmd(nc, in_maps, list(range(N_CORES)))
    out = np.concatenate([res.results[c]["out"] for c in range(N_CORES)], axis=0)
    return out.astype(np.float32)


# revision 46
# speedup vs baseline: 1.3455x; 1.3455x over previous
"""CSPN accelerate (3x3 per-pixel dynamic filter) on 8 Trainium2 NeuronCores.

out[b,0,h,w] = sum_{di,dj in 0..2} K[b,3*di+dj,h,w] * Xpad[b, h+di-1, w+dj-1]
with the center tap (di=dj=1) taking input0 instead of input.

Sharding: pure data-parallel over batch (16 samples -> 2 per core).

The op is memory-bound (11 input planes + 1 output plane per sample, one
multiply-accumulate per loaded element), so the kernel is built around HBM
traffic and DMA-issue efficiency:

  * fp16 everywhere: the host converts inputs to fp16 and upcasts the fp16
    output; products are computed in fp16 with fp32 PSUM accumulation
    (~7e-4 relative absmax error vs the 2e-2 gate). Halves HBM bytes.
  * Each 126-row tile loads with 4 data DMAs: three raw 3-D access
    patterns (one per di) pull the 9 kernel taps with tap (di,dj)
    row-shifted by -di, so every tap product is computed aligned to
    *input* rows; a fourth DMA loads the host-packed, host-zero-padded
    [x, x0] pair. Raw access patterns are only ever used on the DRAM
    (read-only) side -- SBUF operands are plain tile slices so the tile
    framework's dependency tracking sees every byte (an earlier version
    used raw overlapping SBUF window APs, which the tracker cannot see,
    and raced with buffer reuse on real hardware).
  * 4 fused DVE multiplies per tile produce all 9 tap products (one
    [P,3,cn] op per dj with the x slice broadcast across di; the center
    tap multiplies x0).
  * Matmuls against constant shifted-identity matrices realign the row
    (di) shifts and accumulate all 9 taps into PSUM on the Tensor engine;
    ACT copies PSUM (fp32) to SBUF as fp16 for a half-width output DMA.
  * Image edges need NO device-side zero fills: the host pads the xx
    tensor with a zero border (an earlier version zeroed the pad column
    with a 1-elem-per-partition Pool memset, which intermittently lost
    the race against its consumers on real hardware); the top tile's
    out-of-range kernel rows read in-bounds garbage from the preceding
    plane (finite, multiplied by 0 in the shift-matmuls -- the shift
    matrices have no 1s there); the bottom tile's loads and ops are
    partition-clamped to the valid range.
  * The final row-tile is split into 384+256 column chunks so the
    pipeline drain (the last chunk's serial products->matmul->copy->store
    chain) is short; each tile's output store is emitted one tile late so
    its sequencer wait never blocks the next tile's loads, and the
    drain-time stores issue from SP so they cannot block the final
    PSUM->SBUF copy on ACT.

This reads every HBM byte exactly once (plus a 2/126 row halo and a
2-column chunk halo), at half width, and writes the output at half width.
"""

import numpy as np

import concourse.bacc as bacc
import concourse.bass as bass
import concourse.mybir as mybir
import concourse.tile as tile
from concourse.ap import AP
from concourse.bass_utils import run_bass_kernel_spmd

F16 = mybir.dt.float16
F32 = mybir.dt.float32

BS, KK, H, W = 16, 9, 480, 640
N_CORES = 8
BPC = BS // N_CORES          # samples per core
P = 128                      # SBUF partitions
TH = P - 2                   # output rows per tile (input tile covers TH+2 rows)
# Column-chunk splits for the final row-tiles (one tuple per tile, applied
# to the last len(TAIL_SPLITS) tiles). Chunks of >=256 fp16 columns keep
# DMA descriptors >=512B (full-rate).
TAIL_SPLITS = [(384, 256)]
KBUFS, PBUFS, OBUFS = 4, 4, 4
TAIL_POOL_OPS = 0   # >0 runs the final item's center product on Pool
KT_ENGS = ("sync", "scalar", "gpsimd")  # issuing engine per kt di-DMA
XX_ENG = "sync"
SP_FLUSH_TAIL = 2  # flush the last K items' stores from SP instead of ACT


def build_module() -> bass.Bass:
    nc = bacc.Bacc()
    k_ext = nc.declare_dram_parameter("kern", [BPC, KK, H, W], F16, isOutput=False)
    xx_ext = nc.declare_dram_parameter("xx", [BPC, 2, H + 2, W + 2], F16, isOutput=False)
    out_ext = nc.declare_dram_parameter("out", [BPC, 1, H, W], F16, isOutput=True)

    # Shift matrices: M[di][g, h] = 1 iff g == h + di, so that
    # out[h] = sum_g M[g,h] * prod[g] picks input-row-aligned products back
    # into output rows.
    m_np = np.zeros((P, 3, TH), np.float16)
    for di in range(3):
        for h in range(TH):
            m_np[h + di, di, h] = 1.0
    m_dram = nc.inline_tensor(m_np, name="shiftm")

    row_tiles = [(r, min(TH, H - r)) for r in range(0, H, TH)]

    with tile.TileContext(nc) as tc:
        with (
            tc.tile_pool(name="consts", bufs=1) as cpool,
            tc.tile_pool(name="kpool", bufs=KBUFS) as kpool,
            tc.tile_pool(name="prodpool", bufs=PBUFS) as ppool,
            tc.tile_pool(name="opool", bufs=OBUFS) as opool,
            tc.tile_pool(name="psum", bufs=4, space="PSUM") as psumpool,
        ):
            mtile = cpool.tile([P, 3, TH], F16)

            # (b, r, th, c0, cn) work items
            items = []
            n_full = BPC * len(row_tiles) - len(TAIL_SPLITS)
            idx = 0
            for b in range(BPC):
                for r, th in row_tiles:
                    if idx < n_full:
                        items.append((b, r, th, 0, W))
                    else:
                        c0 = 0
                        for cn in TAIL_SPLITS[idx - n_full]:
                            items.append((b, r, th, c0, cn))
                            c0 += cn
                    idx += 1

            pending_out = None
            for i, (b, r, th, c0, cn) in enumerate(items):
                pending_out = _emit_tile(
                    nc, kpool, ppool, opool, psumpool, mtile,
                    k_ext, xx_ext, out_ext, b, r, th, c0, cn,
                    pending_out,
                    # the shift matrices aren't needed until the first
                    # matmul; loading them after the first tile's loads
                    # keeps them off the DMA ramp-in
                    mload=(m_dram if i == 0 else None),
                    last=(i == len(items) - 1),
                    # near the drain, a pending store's sequencer wait on
                    # ACT would delay the next item's ACT-issued kernel
                    # load; route tail stores through SP
                    flush_eng=(nc.sync if i >= len(items) - SP_FLUSH_TAIL
                               else None),
                )
            _flush_out(nc, out_ext, pending_out, eng=nc.sync)
    nc.finalize()
    return nc


def _flush_out(nc, out_ext, pending_out, eng=None):
    if pending_out is not None:
        osb, b, r, th, c0, cn = pending_out
        (eng or nc.scalar).dma_start(
            out=out_ext[b, 0, r:r + th, c0:c0 + cn], in_=osb[:th, 0:cn])


def _kt_src_ap(k_ext, b, r, c0, cn, pl, di):
    """Raw DRAM AP [p, dj, w] -> kern[b, 3*di+dj, r - di + p, c0 + w].

    Read-only DRAM side, so dependency tracking is irrelevant. For the
    top tile (r=0) the di>=1 taps read the tail of the preceding plane
    (in bounds: finite garbage the shift-matmuls multiply by 0), and the
    bottom tile keeps every address in bounds via pl <= th+2.
    """
    hw = H * W
    off = ((b * KK + 3 * di) * H + (r - di)) * W + c0
    hi = off + (pl - 1) * W + 2 * hw + (cn - 1)
    assert 0 <= off and hi < BPC * KK * hw, (b, r, c0, cn, pl, di, off, hi)
    return AP(k_ext[:].tensor, off, [[W, pl], [hw, 3], [1, cn]])


def _emit_tile(nc, kpool, ppool, opool, psumpool, mtile,
               k_ext, xx_ext, out_ext, b, r, th, c0, cn,
               pending_out, mload=None, last=False, flush_eng=None):
    cw = cn + 2              # tile columns cover image cols [c0-1, c0+cn+1)
    pl = min(P, th + 2)      # partitions loaded/used (clamped bottom tile)

    # --- kernel tile [P, di, dj, w]: one DMA per di via raw shifted APs
    # (the DMA lowering balances at most 3 dims per side), spread across
    # the SP / ACT (HWDGE) and Pool (SWDGE) issue paths
    kt = kpool.tile([P, 3, 3, W], F16)
    for di in range(3):
        getattr(nc, KT_ENGS[di]).dma_start(
            out=kt[0:pl, di, :, 0:cn],
            in_=_kt_src_ap(k_ext, b, r, c0, cn, pl, di))

    # --- x/x0 pair tile [P, plane, col]: partition p holds image row
    # r-1+p. The host pads the xx tensor with zero border rows/columns
    # (conv zero-padding), so every tile -- edge or interior -- is one
    # uniform unclamped load with no device-side zero fills at all. (An
    # earlier version zeroed the pad column with a 1-element-per-partition
    # Pool memset, which intermittently lost the race against its
    # consumers on real hardware.)
    xxt = kpool.tile([P, 2, W + 2], F16, tag="xxt")
    getattr(nc, XX_ENG).dma_start(
        out=xxt[0:pl, :, 0:cw],
        in_=xx_ext[b, :, r:r + pl, c0:c0 + cw].transpose([1, 0, 2]),
    )

    if mload is not None:
        nc.sync.dma_start(out=mtile[:], in_=mload[:])

    # --- previous tile's output store (see module docstring)
    _flush_out(nc, out_ext, pending_out, eng=flush_eng)

    # --- fused per-dj product batches on DVE:
    # prod[p, dj, di, w] = kt[p, di, dj, w] * x[p, dj + w] with the x
    # slice broadcast across di; the center tap (1,1) multiplies x0.
    # Every operand is a plain tile slice or a broadcast_to -- APs the
    # tile framework's dependency tracker fully understands -- so buffer
    # rotation is race-free by construction (an earlier version used raw
    # overlapping-window SBUF APs, which the tracker cannot see, and
    # raced with the next tile's loads on real hardware).
    # On the final item the center product runs on the otherwise-idle
    # Pool engine, shortening the serial product chain in the drain.
    prod = ppool.tile([P, 3, 3, W], F16)
    for dj in (0, 2):
        nc.vector.tensor_tensor(
            out=prod[0:pl, dj, :, 0:cn],
            in0=kt[0:pl, :, dj, 0:cn],
            in1=xxt[0:pl, 0, dj:dj + cn].unsqueeze(1).broadcast_to([pl, 3, cn]),
            op=mybir.AluOpType.mult,
        )
    cent_eng = nc.gpsimd if (last and TAIL_POOL_OPS >= 1) else nc.vector
    nc.vector.tensor_tensor(
        out=prod[0:pl, 1, 0::2, 0:cn],
        in0=kt[0:pl, 0::2, 1, 0:cn],
        in1=xxt[0:pl, 0, 1:1 + cn].unsqueeze(1).broadcast_to([pl, 2, cn]),
        op=mybir.AluOpType.mult,
    )
    cent_eng.tensor_tensor(
        out=prod[0:pl, 1, 1, 0:cn],
        in0=kt[0:pl, 1, 1, 0:cn],
        in1=xxt[0:pl, 1, 1:1 + cn],
        op=mybir.AluOpType.mult,
    )

    # --- row-shift realignment + 9-tap accumulation in PSUM on PE
    psum_t = psumpool.tile([P, W], F32)
    for t, (di, dj) in enumerate((di, dj) for di in range(3) for dj in range(3)):
        for m0 in range(0, cn, 512):
            mn = min(512, cn - m0)
            nc.tensor.matmul(
                out=psum_t[:th, m0:m0 + mn],
                lhsT=mtile[0:pl, di, 0:th],
                rhs=prod[0:pl, dj, di, m0:m0 + mn],
                start=(t == 0),
                stop=(t == 8),
            )

    # --- PSUM (fp32) -> SBUF fp16 (on ACT); the store is emitted during
    # the next tile
    osb = opool.tile([P, W], F16, tag="osb")
    nc.scalar.copy(out=osb[:th, 0:cn], in_=psum_t[:th, 0:cn])
    return (osb, b, r, th, c0, cn)


def pack_xx(x, x0):
    """fp16 [n, 2, H+2, W+2]: x and x0 planes with a zero border."""
    n = x.shape[0]
    xx = np.zeros((n, 2, H + 2, W + 2), np.float16)
    xx[:, 0, 1:H + 1, 1:W + 1] = x[:, 0]
    xx[:, 1, 1:H + 1, 1:W + 1] = x0[:, 0]
    return xx


_NC_CACHE = None


def _get_module():
    global _NC_CACHE
    if _NC_CACHE is None:
        _NC_CACHE = build_module()
    return _NC_CACHE


def kernel(**inputs: np.ndarray) -> np.ndarray:
    kern = np.asarray(inputs["kernel"], dtype=np.float32).astype(np.float16)
    x = np.asarray(inputs["input"], dtype=np.float32).astype(np.float16)
    x0 = np.asarray(inputs["input0"], dtype=np.float32).astype(np.float16)
    assert kern.shape == (BS, KK, H, W), kern.shape

    xx = pack_xx(x, x0)
    nc = _get_module()
    in_maps = [
        {"kern": np.ascontiguousarray(kern[c * BPC:(c + 1) * BPC]),
         "xx": np.ascontiguousarray(xx[c * BPC:(c + 1) * BPC])}
        for c in range(N_CORES)
    ]
    res = run_bass_kernel_spmd(nc, in_maps, list(range(N_CORES)))
    out = np.concatenate([res.results[c]["out"] for c in range(N_CORES)], axis=0)
    return out.astype(np.float32)
